# revision 30
# baseline (speedup 1.0000x reference)
"""Causal self-attention (RoPE, 16 heads) on 8 Trainium2 NeuronCores.

Sharding: data parallel over batch (2) x tensor parallel over head groups
(16 heads -> 4 groups of 4). Core c handles batch c//4, head group c%4.
Each core computes q/k/v projections for its 4 heads, RoPE, causal
softmax(q k^T / sqrt(d)) v, and its slice of the output projection; the
host sums the 4 tensor-parallel partials per batch.

Layouts (per core):
  xT [1024 D, 2048 S]   q/k transposed [256 ch, 2048 S] (head dim on
  partitions, so scores need no transposes), v natural [2048 S, 4, 64+1]
  with a ones column so attn@v also produces the softmax denominators.
  Scores are computed transposed S[k, q]; attn@v uses the exp tiles as
  the stationary operand giving o in natural [q, ch] layout, where the
  denominator lands in a psum column -> per-partition reciprocal +
  tensor_scalar normalize. o is then PE-transposed back to [ch, q] for
  the output projection. fp16 operands, fp32 psum accumulation.
"""
import numpy as np

import concourse.bass as bass
import concourse.mybir as mybir
import concourse.tile as tile
from concourse.vector_clock import ScopedClock
from concourse.bass_utils import run_bass_kernel_spmd

F32 = mybir.dt.float32
F16 = mybir.dt.float16

D_MODEL = 1024
N_HEADS = 16
HEAD_DIM = 64
SEQ = 2048
BATCH = 2
N_CORES = 8
HEADS_PER_CORE = 4
GROUPS = 4
CH = HEADS_PER_CORE * HEAD_DIM  # 256

MAX_WAITS = 1


def _cap_waits(nc: bass.Bass, cap: int):
    """walrus here only accepts `cap` sem waits per instruction; hoist the
    overflow onto same-engine nops inserted just before."""
    nid = [0]

    def mknop(engine, waits):
        nid[0] += 1
        n = mybir.InstNoOp(name=f"I-waitcap-{nid[0]}", ins=[], outs=[])
        n.engine = engine
        n.sync_info = mybir.SyncInfo(on_wait=list(waits), on_update=[])
        return n

    for fn in nc.m.functions:
        for bb in fn.blocks:
            out = []
            changed = False
            for ins in bb.instructions:
                si = ins.sync_info
                w = list(si.on_wait) if si and si.on_wait else []
                if len(w) > cap:
                    changed = True
                    keep = w[-cap:]
                    rest = w[: len(w) - cap]
                    eng = ins.engine
                    if eng == mybir.EngineType.Unassigned:
                        eng = mybir.EngineType.SP
                    for i in range(0, len(rest), cap):
                        out.append(mknop(eng, rest[i : i + cap]))
                    si.on_wait = keep
                out.append(ins)
            if changed:
                bb.instructions = out


class KTileContext(tile.TileContext):
    def _drain_and_barrier(self, tick_clock, wait_clock):
        drain_inst = self.nc.sync.drain()
        wait_clock.add_sem_waits(
            drain_inst.ins, ScopedClock({None: tick_clock.global_clock})
        )
        si = drain_inst.ins.sync_info
        w = si.on_wait if si else None
        if w and len(w) > 1:
            si.on_wait = []
            for sw in w:
                n2 = self.nc.sync.nop()
                if n2.ins.sync_info is None:
                    n2.ins.sync_info = mybir.SyncInfo(on_wait=[sw], on_update=[])
                else:
                    n2.ins.sync_info.on_wait = [sw]
            self.nc.sync.drain()
        self.nc.all_engine_barrier()
        assert self.sems is not None
        popped = self.nc._tile_sem_poison_stack.pop()
        assert popped is self._sem_poison
        self.nc.clear_and_free_semaphores(list(self.sems.allocated().values()))
        self.nc.all_engine_barrier()

    def __exit__(self, exc_type, exc_value, traceback):
        r = super().__exit__(exc_type, exc_value, traceback)
        if exc_type is None:
            _cap_waits(self.nc, MAX_WAITS)
        return r


def build_program() -> bass.Bass:
    nc = bass.Bass()

    xt_d = nc.dram_tensor("xt", [D_MODEL, SEQ], F16, kind="ExternalInput")
    wq_d = nc.dram_tensor("wq", [D_MODEL, CH], F16, kind="ExternalInput")
    wk_d = nc.dram_tensor("wk", [D_MODEL, CH], F16, kind="ExternalInput")
    wv_d = nc.dram_tensor("wv", [D_MODEL, CH], F16, kind="ExternalInput")
    wo_d = nc.dram_tensor("wo", [CH, D_MODEL], F16, kind="ExternalInput")
    cos_d = nc.dram_tensor("cos2", [64, SEQ], F16, kind="ExternalInput")
    sin_d = nc.dram_tensor("sin2", [64, SEQ], F16, kind="ExternalInput")
    msk_d = nc.dram_tensor("msk", [128, 2 * 512], F16, kind="ExternalInput")
    rot_d = nc.dram_tensor("rot", [128, 128], F16, kind="ExternalInput")
    idn_d = nc.dram_tensor("idn", [128, 128], F16, kind="ExternalInput")
    out_d = nc.dram_tensor("out", [SEQ, D_MODEL], F16, kind="ExternalOutput")

    NQ = SEQ // 512       # 4 q chunks of 512
    NROW = SEQ // 128     # 16 row chunks / q tiles
    KD = D_MODEL // 128   # 8 contraction chunks

    with KTileContext(nc) as tc, nc.allow_low_precision(reason="fp16 pipeline"):
        with (
            tc.tile_pool(name="wgt", bufs=1) as wgt,
            tc.tile_pool(name="tabs", bufs=1) as tabs,
            tc.tile_pool(name="qk", bufs=1) as qkp,
            tc.tile_pool(name="vp", bufs=1) as vp,
        ):
            wq_sb = [wgt.tile([128, CH], F16, name=f"wq{k}", tag=f"wq{k}") for k in range(KD)]
            wk_sb = [wgt.tile([128, CH], F16, name=f"wk{k}", tag=f"wk{k}") for k in range(KD)]
            wv_sb = [wgt.tile([128, CH], F16, name=f"wv{k}", tag=f"wv{k}") for k in range(KD)]
            cos_sb = tabs.tile([128, SEQ], F16, tag="cos")
            sin_sb = tabs.tile([128, SEQ], F16, tag="sin")
            rot_sb = tabs.tile([128, 128], F16, tag="rot")
            idn_sb = tabs.tile([128, 128], F16, tag="idn")
            q_sb = [qkp.tile([128, SEQ], F16, name=f"q{t}", tag=f"q{t}") for t in range(2)]
            k_sb = [qkp.tile([128, SEQ], F16, name=f"k{t}", tag=f"k{t}") for t in range(2)]
            v_sb = [vp.tile([128, HEADS_PER_CORE, 65], F16, name=f"v{r}", tag=f"v{r}")
                    for r in range(NROW)]

            for r in range(NROW):
                nc.vector.memset(v_sb[r][:, :, 64:65], 1.0)

            # ---------------- phase 1: projections + RoPE ----------------
            with (
                tc.tile_pool(name="xt", bufs=1) as xtp,
                tc.tile_pool(name="praw", bufs=2) as praw,
                tc.tile_pool(name="pp", bufs=3, space="PSUM") as pp,
                tc.tile_pool(name="rp", bufs=2, space="PSUM") as rp,
            ):
                xt_sb = [xtp.tile([128, SEQ], F16, name=f"xt{k}", tag=f"xt{k}") for k in range(KD)]
                nc.sync.dma_start(out=wv_sb[0][:], in_=wv_d[0:128, :])
                for k in range(KD):
                    for j in range(4):
                        nc.sync.dma_start(
                            out=xt_sb[k][:, j * 512:(j + 1) * 512],
                            in_=xt_d[k * 128:(k + 1) * 128, j * 512:(j + 1) * 512])
                    if k > 0:
                        nc.sync.dma_start(out=wv_sb[k][:], in_=wv_d[k * 128:(k + 1) * 128, :])
                    nc.gpsimd.dma_start(out=wq_sb[k][:], in_=wq_d[k * 128:(k + 1) * 128, :])
                    nc.gpsimd.dma_start(out=wk_sb[k][:], in_=wk_d[k * 128:(k + 1) * 128, :])
                nc.gpsimd.dma_start(out=cos_sb[0:64, :], in_=cos_d[:])
                nc.gpsimd.dma_start(out=cos_sb[64:128, :], in_=cos_d[:])
                nc.gpsimd.dma_start(out=sin_sb[0:64, :], in_=sin_d[:])
                nc.gpsimd.dma_start(out=sin_sb[64:128, :], in_=sin_d[:])
                nc.gpsimd.dma_start(out=rot_sb[:], in_=rot_d[:])
                nc.gpsimd.dma_start(out=idn_sb[:], in_=idn_d[:])

                # v projection: natural layout [rows, 256]
                for r in range(NROW):
                    ps = pp.tile([128, CH], F32, tag="pv")
                    for k in range(KD):
                        nc.tensor.matmul(
                            ps[:], xt_sb[k][:, r * 128:(r + 1) * 128], wv_sb[k][:],
                            start=(k == 0), stop=(k == KD - 1))
                    for h in range(HEADS_PER_CORE):
                        nc.scalar.copy(
                            v_sb[r][:, h, 0:64], ps[:, h * 64:(h + 1) * 64])

                # q/k projections transposed [channels, rows] + RoPE
                for m in range(2):                # 128-channel chunks (2 heads)
                    for which, w_sb, dst in (("q", wq_sb, q_sb), ("k", wk_sb, k_sb)):
                        raw = praw.tile([128, SEQ], F16, tag="raw")
                        for n in range(NQ):       # 512-row chunks
                            ps = pp.tile([128, 512], F32, tag="pqk")
                            for k in range(KD):
                                nc.tensor.matmul(
                                    ps[:],
                                    w_sb[k][:, m * 128:(m + 1) * 128],
                                    xt_sb[k][:, n * 512:(n + 1) * 512],
                                    start=(k == 0), stop=(k == KD - 1))
                            nc.scalar.copy(raw[:, n * 512:(n + 1) * 512], ps[:])
                        for n in range(NQ):
                            sl = slice(n * 512, (n + 1) * 512)
                            pr = rp.tile([128, 512], F32, tag="prot")
                            nc.tensor.matmul(pr[:], rot_sb[:], raw[:, sl],
                                             start=True, stop=True)
                            t1 = praw.tile([128, 512], F16, tag="t1")
                            nc.vector.tensor_mul(t1[:], raw[:, sl], cos_sb[:, sl])
                            t2 = praw.tile([128, 512], F16, tag="t2")
                            nc.vector.tensor_mul(t2[:], pr[:], sin_sb[:, sl])
                            nc.vector.tensor_add(dst[m][:, sl], t1[:], t2[:])

            # ---------------- phase 2: attention ----------------
            with (
                tc.tile_pool(name="att", bufs=1) as att,
                tc.tile_pool(name="se", bufs=5) as sep,
                tc.tile_pool(name="nrm", bufs=4) as nrm,
                tc.tile_pool(name="onat", bufs=6) as onp,
                tc.tile_pool(name="ow", bufs=1) as owp,
                tc.tile_pool(name="outp", bufs=4) as outp,
            ):
                msk_sb = att.tile([128, 2 * 512], F16, tag="msk")
                nc.sync.dma_start(out=msk_sb[:], in_=msk_d[:])
                wo_sb = [owp.tile([128, D_MODEL], F16, name=f"wo{k}", tag=f"wo{k}")
                         for k in range(2)]
                for k in range(2):
                    nc.sync.dma_start(out=wo_sb[k][:], in_=wo_d[k * 128:(k + 1) * 128, :])
                oT = [att.tile([128, SEQ], F16, name=f"oT{t}", tag=f"oT{t}") for t in range(2)]

                with (
                    tc.tile_pool(name="pso", bufs=2, space="PSUM") as pso,
                    tc.tile_pool(name="psa", bufs=2, space="PSUM") as psa,
                    tc.tile_pool(name="psm", bufs=2, space="PSUM") as psm,
                ):
                    for qc in range(NQ):              # q chunk of 512
                        qs0 = qc * 512
                        nkt = 4 * qc + 4              # causal k tiles
                        for th in range(2):           # head pair
                            onat_tiles = {
                                qt4: onp.tile([128, 2, 64], F16,
                                              name=f"on{th}_{4 * qc + qt4}",
                                              tag="onat")
                                for qt4 in range(4)
                            }
                            po = [psa.tile([128, HEADS_PER_CORE, 65], F32,
                                           name=f"po{th}_{qc}_{hh}", tag="po")
                                  for hh in range(2)]
                            for hh in range(2):
                                # init: overwrite whole blob with zeros so the
                                # four interleaved accumulation groups never
                                # issue a start=True (which would clear the
                                # whole bank's has_written bits mid-flight)
                                nc.tensor.matmul(
                                    po[hh][:].rearrange("p a b -> p (a b)"),
                                    msk_sb[0:1, 0:128],
                                    msk_sb[0:1, 512:512 + 4 * 65],
                                    start=True, stop=True)
                            for kt in range(nkt):
                                rel = kt - 4 * qc
                                # columns left of the diagonal sub-chunk are
                                # fully masked and never consumed: skip them
                                c0 = max(rel, 0) * 128
                                pss = pso.tile([128, 2, 512], F32, tag="ps")
                                for hh in range(2):
                                    b0 = 64 * hh
                                    nc.tensor.matmul(
                                        pss[:, hh, c0:512],
                                        k_sb[th][b0:b0 + 64, kt * 128:(kt + 1) * 128],
                                        q_sb[th][b0:b0 + 64, qs0 + c0:qs0 + 512],
                                        start=True, stop=True)
                                s = sep.tile([128, 2, 512], F16, tag="se")
                                nc.scalar.activation(
                                    s[:, :, c0:512], pss[:, :, c0:512],
                                    mybir.ActivationFunctionType.Exp, scale=0.125)
                                if rel >= 0:
                                    # triangular mask on the diagonal sub-chunk
                                    for hh in range(2):
                                        nc.vector.tensor_mul(
                                            s[:, hh, c0:c0 + 128],
                                            s[:, hh, c0:c0 + 128],
                                            msk_sb[:, 0:128])
                                for hh in range(2):
                                    head = 2 * th + hh
                                    for qt4 in range(4):
                                        gq = 4 * qc + qt4
                                        if kt > gq:
                                            continue
                                        nc.tensor.matmul(
                                            po[hh][:, qt4, :],
                                            s[:, hh, qt4 * 128:(qt4 + 1) * 128],
                                            v_sb[kt][:, head, :],
                                            start=False, stop=(kt == gq),
                                            skip_group_check=True)
                            # normalize -> o natural [q, ch] fp16
                            for hh in range(2):
                                rcol = nrm.tile([128, 4, 1], F32, tag="rcol")
                                nc.vector.reciprocal(rcol[:], po[hh][:, :, 64:65])
                                for qt4 in range(4):
                                    nc.vector.tensor_scalar_mul(
                                        onat_tiles[qt4][:, hh, :],
                                        po[hh][:, qt4, 0:64],
                                        rcol[:, qt4, :])
                            # transpose o natural -> oT [ch, q]
                            for qt4 in range(4):
                                gq = 4 * qc + qt4
                                pt = psm.tile([128, 128], F16, name=f"pt{th}_{gq}",
                                              tag="misc")
                                nc.tensor.transpose(
                                    pt[:],
                                    onat_tiles[qt4][:].rearrange("p a b -> p (a b)"),
                                    idn_sb[:])
                                nc.vector.tensor_copy(
                                    oT[th][:, gq * 128:(gq + 1) * 128], pt[:])

                        # ---- output projection for this q chunk ----
                        for qt4 in range(4):
                            qt = 4 * qc + qt4
                            for nn in range(2):
                                pf = psm.tile([128, 512], F32,
                                              name=f"pf{qt}_{nn}", tag="misc")
                                for k in range(2):
                                    nc.tensor.matmul(
                                        pf[:],
                                        oT[k][:, qt * 128:(qt + 1) * 128],
                                        wo_sb[k][:, nn * 512:(nn + 1) * 512],
                                        start=(k == 0), stop=(k == 1))
                                ob = outp.tile([128, 512], F16, tag="ob")
                                if nn == 0:
                                    nc.scalar.copy(ob[:], pf[:])
                                else:
                                    nc.vector.tensor_copy(ob[:], pf[:])
                                nc.sync.dma_start(
                                    out=out_d[qt * 128:(qt + 1) * 128,
                                              nn * 512:(nn + 1) * 512],
                                    in_=ob[:])
    return nc


_PROGRAM_CACHE = {}


def _get_program():
    if "nc" not in _PROGRAM_CACHE:
        _PROGRAM_CACHE["nc"] = build_program()
    return _PROGRAM_CACHE["nc"]


def _host_inputs(x, cos, sin, Wq, Wk, Wv, Wo):
    f16 = np.float16
    cosT = np.ascontiguousarray(cos.T)
    sinT = np.ascontiguousarray(sin.T)
    cos2 = np.tile(cosT, (2, 1)).astype(f16)
    sin2 = np.tile(sinT, (2, 1)).astype(f16)

    R = np.zeros((HEAD_DIM, HEAD_DIM), np.float32)
    R[np.arange(32), np.arange(32) + 32] = -1.0
    R[np.arange(32) + 32, np.arange(32)] = 1.0
    RT = R.T
    rot = np.zeros((128, 128), np.float32)
    rot[0:64, 0:64] = RT
    rot[64:128, 64:128] = RT
    rot = rot.astype(f16)

    msk = np.zeros((128, 2 * 512), np.float32)
    p = np.arange(128)[:, None]
    f = np.arange(512)[None, :]
    msk[:, 0:512] = (p - f <= 0)          # triangular block; cols 512: zeros
    msk = msk.astype(f16)

    idn = np.eye(128, dtype=f16)

    in_maps = []
    for c in range(N_CORES):
        b, g = divmod(c, GROUPS)
        rows = slice(g * CH, (g + 1) * CH)
        in_maps.append({
            "xt": np.ascontiguousarray(x[b].T).astype(f16),
            "wq": np.ascontiguousarray(Wq[rows, :].T).astype(f16),
            "wk": np.ascontiguousarray(Wk[rows, :].T).astype(f16),
            "wv": np.ascontiguousarray(Wv[rows, :].T).astype(f16),
            "wo": np.ascontiguousarray(Wo[:, rows].T).astype(f16),
            "cos2": cos2, "sin2": sin2, "msk": msk, "rot": rot, "idn": idn,
        })
    return in_maps


def kernel(x, cos, sin, Wq, Wk, Wv, Wo, _trace=False, _trace_kwargs=None):
    nc = _get_program()
    in_maps = _host_inputs(x, cos, sin, Wq, Wk, Wv, Wo)
    kw = {}
    if _trace:
        kw["trace"] = True
        if _trace_kwargs:
            kw.update(_trace_kwargs)
    res = run_bass_kernel_spmd(nc, in_maps, list(range(N_CORES)), **kw)
    out = np.zeros((BATCH, SEQ, D_MODEL), np.float32)
    for c in range(N_CORES):
        b = c // GROUPS
        out[b] += res.results[c]["out"].astype(np.float32)
    kernel.last_result = res
    return out
